# revision 17
# baseline (speedup 1.0000x reference)
"""BilateralRotation Trainium2 kernel: out[b,c] = R1[c] @ wkv[b,c] @ R2[c],
R = Cayley(p) = (I - A)(I + A)^-1, A = 0.5(p - p^T).

Sharding: 8 NeuronCores, head-parallel - core k owns heads [4k, 4k+4) for all
512 batches. bf16 end-to-end on the data path (rel-err budget 2e-2; measured
~4.5e-3): the host converts the wkv shard to bf16 in a [c, i, b, j] layout so
every DMA line is a 2KB contiguous run, and the device writes bf16 results
that the host casts back to fp32.

Device program per core:
  Phase 1 - Cayley via Newton-Schulz in fp32, 4 pair-packed lanes
    (two 64x64 matrices stacked on 128 partitions; block-diagonal stationaries
    so each NS step is ONE matmul per product):
      B = A^T A, M = I + B, X' = 2X - X(MX), 12 iters, X0 = I/300.
      X is symmetric (polynomial in M), so no X^T tracking is needed.
      R1 path stores R^T = X C^T, R2 path stores R = C X (C^T = I + 2A - B).
  Phase 2 - main loop over 32 groups of 16 batches, all-bf16:
    - MM1: K=128 block-diag stationary blockdiag(R1_c0^T, R1_c1^T) per head
      pair; moving = xin [128, 512] -> Y pair-stacked in PSUM (1 col/cycle,
      full PE).
    - T1: PE transposes of bf16 [128,128] blocks (1 cyc/row vs 2 for fp32).
    - MM2: stationary blockdiag(R2_c, R2_c) (batch-pair K packing), moving
      gathers head c' columns via a strided AP, N=512.
    - PSUM->SBUF copies split across DVE / Act / Pool engines.
    - zsb [128, 2048] bf16 dumped contiguously; host inverts the fixed index
      permutation while unsharding.
"""

import sys
import types
from contextlib import ExitStack

import numpy as np
import ml_dtypes

# ---------------------------------------------------------------------------
# TileContext patch: this walrus build accepts only ONE sync-wait per
# instruction; hoist extra waits onto nops inserted before the instruction.
# ---------------------------------------------------------------------------
import concourse.bass as bass
import concourse.tile as tile
from concourse.vector_clock import ScopedClock
from concourse import masks, mybir
from concourse.bass_utils import run_bass_kernel_spmd

WAIT_LIMIT = 1


def _hoist_extra_waits(nc, inst, hint):
    nops = []
    si = inst.sync_info
    if si is not None and len(si.on_wait) > WAIT_LIMIT:
        extras = si.on_wait[:-WAIT_LIMIT]
        del si.on_wait[:-WAIT_LIMIT]
        for w in extras:
            nop = nc.engines[inst.engine].nop(nofuse=True, hint=hint)
            nsi = nop.ins.sync_info
            if nsi is None:
                nop.ins.sync_info = mybir.SyncInfo(on_wait=[w], on_update=[])
            else:
                nsi.on_wait.append(w)
            nops.append(nop.ins)
    return nops


def _split_waits(nc):
    cur_list = nc.cur_bb.bb.instructions
    for f in nc.m.functions:
        for bb in f.blocks:
            orig = list(bb.instructions)
            if not any(i.sync_info and len(i.sync_info.on_wait) > WAIT_LIMIT
                       for i in orig):
                continue
            new_list = []
            for inst in orig:
                nops = _hoist_extra_waits(nc, inst, "split_wait")
                for nop in nops:
                    if cur_list and cur_list[-1] is nop:
                        cur_list.pop()
                    else:
                        cur_list.remove(nop)
                new_list.extend(nops)
                new_list.append(inst)
            bb.instructions[:] = new_list


def _drain_and_barrier(self, tick_clock, wait_clock):
    nc = self.nc
    _split_waits(nc)
    drain_inst = nc.sync.drain()
    wait_clock.add_sem_waits(drain_inst.ins,
                             ScopedClock({None: tick_clock.global_clock}))
    nops = _hoist_extra_waits(nc, drain_inst.ins, "drain_split_wait")
    if nops:
        insts = nc.cur_bb.bb.instructions
        di = insts.index(drain_inst.ins)
        insts.append(insts.pop(di))
    nc.all_engine_barrier()
    assert self.sems is not None
    popped = nc._tile_sem_poison_stack.pop()
    assert popped is self._sem_poison
    nc.clear_and_free_semaphores(list(self.sems.allocated().values()))
    nc.all_engine_barrier()


tile.TileContext._drain_and_barrier = _drain_and_barrier

# ---------------------------------------------------------------------------
# Program builder
# ---------------------------------------------------------------------------
dt = mybir.dt
F32 = dt.float32
BF16 = dt.bfloat16
BF_NP = np.dtype(ml_dtypes.bfloat16)

HPC = 4                     # heads per core
B = 512
H = W = 64
NG = 32                     # batch groups of 16
GB = 16                     # batches per group
N_CORES = 8
NS_ITERS = 12
C0 = 1.0 / 300.0



def _ecopy(eng, dst, src):
    if hasattr(eng, "tensor_copy"):
        eng.tensor_copy(dst, src)
    else:
        eng.copy(dst, src)


def build(in_bufs=12, out_bufs=4, mm2_delay=8):
    nc = bass.Bass("TRN2", target_bir_lowering=False, debug=False,
                   num_devices=N_CORES)
    # [c, i, b, j] bf16 layout: partition lines are 2KB contiguous runs.
    wkv = nc.dram_tensor("wkv", [HPC, H, B, W], BF16, kind="ExternalInput")
    p_left = nc.dram_tensor("p_left", [HPC, H, H], F32, kind="ExternalInput")
    p_right = nc.dram_tensor("p_right", [HPC, W, W], F32,
                             kind="ExternalInput")
    out = nc.dram_tensor("out_scr", [NG, 128, 2048], BF16,
                         kind="ExternalOutput")
    F16 = dt.float16
    NS_F16 = 0                  # fp16 NS disabled pending debug

    with tile.TileContext(nc) as tc, ExitStack() as ctx:
        const_pool = ctx.enter_context(tc.tile_pool(name="const", bufs=1))
        bd_pool = ctx.enter_context(tc.tile_pool(name="bd", bufs=1))
        ns_sb = ctx.enter_context(tc.tile_pool(name="ns_sb", bufs=2))
        ns_keep = ctx.enter_context(tc.tile_pool(name="ns_keep", bufs=2))
        io_pool = ctx.enter_context(tc.tile_pool(name="io", bufs=in_bufs))
        out_pool = ctx.enter_context(tc.tile_pool(name="outp", bufs=out_bufs))
        mid_pool = ctx.enter_context(tc.tile_pool(name="mid", bufs=3))
        midt_pool = ctx.enter_context(
            tc.tile_pool(name="midt", bufs=mm2_delay + 2))
        ps_pool = ctx.enter_context(
            tc.tile_pool(name="ps", bufs=1, space="PSUM"))

        ident = const_pool.tile([128, 128], F32, tag="ident")
        masks.make_identity(nc, ident[:])
        i64 = ident[0:64, 0:64]
        identb = const_pool.tile([128, 128], BF16, tag="identb")
        nc.vector.tensor_copy(identb[:], ident[:])
        # [I64; I64] stacked
        istk = const_pool.tile([128, 64], F32, tag="istk")
        nc.vector.tensor_copy(istk[0:64, :], i64)
        nc.scalar.copy(istk[64:128, :], i64)

        bdl = []
        for P in range(2):
            t = bd_pool.tile([128, 128], BF16, tag=f"bdl{P}")
            nc.gpsimd.memset(t[:], 0.0)
            bdl.append(t)
        bdr = []
        for c in range(HPC):
            t = bd_pool.tile([128, 128], BF16, tag=f"bdr{c}")
            nc.gpsimd.memset(t[:], 0.0)
            bdr.append(t)

        # ---------------- Newton-Schulz helpers (pair-packed lanes) --------
        # lane 0,1: p_left pairs (R1); lane 2,3: p_right pairs (R2).
        # NS PSUM tiles borrow the MM2 tag: NS is done (slot 6) before the
        # first MM2 is emitted (slot mm2_delay), so the banks time-share.
        def ns_ps_tile():
            return ps_pool.tile([128, 512], F32, tag="mm2", bufs=2,
                                name="nsps")

        def qmm(out_t, s, m):
            nc.tensor.matmul(out_t[0:64, 0:64], s[0:64, :], m[0:64, :])
            nc.tensor.matmul(out_t[64:128, 0:64], s[64:128, :], m[64:128, :])

        def ns_setup(L):
            src_t = p_left if L < 2 else p_right
            ca, cb = 2 * (L % 2), 2 * (L % 2) + 1
            pside = ns_sb.tile([64, 128], F32, tag=f"pside{L}")
            nc.sync.dma_start(pside[:, 0:64], src_t.ap()[ca])
            nc.sync.dma_start(pside[:, 64:128], src_t.ap()[cb])
            pstk = ns_sb.tile([128, 64], F32, tag=f"pstk{L}")
            nc.sync.dma_start(pstk[0:64, :], src_t.ap()[ca])
            nc.sync.dma_start(pstk[64:128, :], src_t.ap()[cb])

            ptp = ns_ps_tile()
            nc.tensor.transpose(ptp[:, 0:64], pside[:], i64)

            astk = ns_keep.tile([128, 64], F32, tag=f"astk{L}")
            nc.vector.tensor_sub(astk[:], pstk[:], ptp[:, 0:64])
            nc.vector.tensor_scalar_mul(astk[:], astk[:], 0.5)

            bps = ns_ps_tile()
            nc.tensor.matmul(bps[0:64, 0:64], astk[0:64, :], astk[0:64, :])
            nc.tensor.matmul(bps[64:128, 0:64], astk[64:128, :],
                             astk[64:128, :])

            mstk = ns_keep.tile([128, 64], F32, tag=f"mstk{L}")
            nc.vector.tensor_add(mstk[:], bps[:, 0:64], istk[:])  # M = I + B
            mstk16 = ns_keep.tile([128, 64], F16, tag=f"mstk16{L}")
            nc.scalar.copy(mstk16[:], mstk[:])
            ctstk = ns_keep.tile([128, 64], F32, tag=f"ct{L}")
            nc.vector.scalar_tensor_tensor(                       # 2A - B
                ctstk[:], astk[:], 2.0, bps[:, 0:64],
                op0=mybir.AluOpType.mult, op1=mybir.AluOpType.subtract)
            nc.vector.tensor_add(ctstk[:], ctstk[:], istk[:])     # C^T
            xstk = ns_keep.tile([128, 64], F32, tag=f"x{L}")
            nc.vector.tensor_scalar_mul(xstk[:], istk[:], C0)
            x16 = ns_keep.tile([128, 64], F16, tag=f"x16{L}")
            nc.scalar.copy(x16[:], xstk[:])
            return dict(x16=x16, xstk=xstk, mstk=mstk, mstk16=mstk16,
                        ctstk=ctstk, it=0)

        def ns_iter_a(L, ln, cpeng):
            fp16 = ln["it"] < NS_F16
            pps = ns_ps_tile()
            if fp16:
                qmm(pps, ln["mstk16"], ln["x16"])                 # M X (sym)
            else:
                qmm(pps, ln["mstk"], ln["xstk"])
            psb = ns_sb.tile([128, 64], F16 if fp16 else F32, tag=f"psb{L}",
                             name=f"psb{L}_{ln['it']}")
            _ecopy(cpeng, psb[:], pps[:, 0:64])
            ln["psb"] = psb

        def ns_iter_b(L, ln):
            fp16 = ln["it"] < NS_F16
            last16 = ln["it"] == NS_F16 - 1
            wps = ns_ps_tile()
            qmm(wps, ln["x16"] if fp16 else ln["xstk"], ln["psb"])
            if fp16 and not last16:
                xnew = ns_keep.tile([128, 64], F16, tag=f"x16{L}")
            else:
                xnew = ns_keep.tile([128, 64], F32, tag=f"x{L}")
            nc.vector.scalar_tensor_tensor(                       # 2X - X(MX)
                xnew[:], (ln["x16"] if fp16 else ln["xstk"])[:], 2.0,
                wps[:, 0:64],
                op0=mybir.AluOpType.mult, op1=mybir.AluOpType.subtract)
            if fp16 and not last16:
                ln["x16"] = xnew
            else:
                ln["xstk"] = xnew
            ln["it"] += 1

        def ns_iter(L, ln, cpeng):
            ns_iter_a(L, ln, cpeng)
            ns_iter_b(L, ln)

        def ns_final(L, ln):
            rs = ns_ps_tile()
            if L < 2:
                # R^T = X C^T (X symmetric) -> bdl blocks
                qmm(rs, ln["xstk"], ln["ctstk"])
                nc.vector.tensor_copy(bdl[L][0:64, 0:64], rs[0:64, 0:64])
                nc.scalar.copy(bdl[L][64:128, 64:128], rs[64:128, 0:64])
            else:
                # R = C X = (C^T)^T X -> bdr blocks (replicated)
                qmm(rs, ln["ctstk"], ln["xstk"])
                ca, cb = 2 * (L - 2), 2 * (L - 2) + 1
                nc.vector.tensor_copy(bdr[ca][0:64, 0:64], rs[0:64, 0:64])
                nc.scalar.copy(bdr[ca][64:128, 64:128], rs[0:64, 0:64])
                nc.vector.tensor_copy(bdr[cb][0:64, 0:64], rs[64:128, 0:64])
                nc.scalar.copy(bdr[cb][64:128, 64:128], rs[64:128, 0:64])

        # ---------------- main-loop pieces ----------------
        PSTRIDE = B * W
        PAIRSTR = 2 * H * PSTRIDE

        def emit_dma_in(g):
            t = io_pool.tile([128, 2048], BF16, tag="xin")
            nc.sync.dma_start(
                t[:], bass.AP(wkv, g * GB * W,
                              [[PSTRIDE, 128], [PAIRSTR, 2], [1, 1024]]))
            return t

        def emit_mm1_t1(g, xin, fill=None):
            # MM1: one [128,1024] PSUM tile per pair, single CAST out.
            # copy split alternates by group parity to balance DVE/Act.
            ysb = [mid_pool.tile([128, 1024], BF16, tag=f"ysb{P}",
                                 name=f"ysb{P}_{g}") for P in range(2)]
            mm1_eng = [nc.vector, nc.scalar] if g % 2 else [nc.scalar,
                                                            nc.scalar]
            for P in range(2):
                yps = ps_pool.tile([128, 1024], F32, tag="mm1", bufs=2)
                for h in range(2):
                    nc.tensor.matmul(
                        yps[:, 512 * h:512 * h + 512], bdl[P][:],
                        xin[:, 1024 * P + 512 * h:1024 * P + 512 * h + 512])
                _ecopy(mm1_eng[P], ysb[P][:], yps[:])
            ytsb = midt_pool.tile([128, 2048], BF16, tag="ytsb",
                                  name=f"ytsb_{g}")
            tps = ps_pool.tile([128, 2048], BF16, tag="t1", bufs=1)
            for P in range(2):
                if fill:
                    fill()
                for q in range(8):
                    nc.tensor.transpose(
                        tps[:, 1024 * P + 128 * q:1024 * P + 128 * q + 128],
                        ysb[P][:, 128 * q:128 * q + 128], identb[:])
            nc.vector.tensor_copy(ytsb[:], tps[:])
            return ytsb

        def emit_mm2(g, ytsb):
            zsb = out_pool.tile([128, 2048], BF16, tag="zsb")
            mm2_eng = ([nc.vector, nc.scalar, nc.scalar, nc.vector] if g % 2
                       else [nc.scalar, nc.vector, nc.scalar, nc.scalar])
            cp = 0
            for P in range(2):
                for cc in range(2):
                    c = 2 * P + cc
                    zps = ps_pool.tile([128, 512], F32, tag="mm2", bufs=2)
                    base = ytsb[:, 1024 * P + 64 * cc:1024 * P + 64 * cc + 64]
                    rhs = bass.AP(base.tensor, base.offset,
                                  [list(base.ap[0]), [128, 8], [1, 64]])
                    nc.tensor.matmul(zps[:], bdr[c][:], rhs)
                    _ecopy(mm2_eng[cp], zsb[:, 512 * c:512 * c + 512], zps[:])
                    cp += 1
            nc.sync.dma_start(
                bass.AP(out, g * 128 * 2048, [[2048, 128], [1, 2048]]),
                zsb[:])

        # ---------------- schedule ----------------
        # Head: R1 NS (lanes 0,1) fully -> bdl. R2 NS (lanes 2,3) is woven
        # two iterations per group slot, each a/b half sitting between PE
        # bursts; MM2 is delayed mm2_delay groups (bdr safe) and the backlog
        # drains 2/slot over the last slots.
        lanes = [ns_setup(L) for L in range(2)]
        for k in range(NS_ITERS):
            for L in range(2):
                ns_iter_a(L, lanes[L], nc.scalar if L == 0 else nc.vector)
            for L in range(2):
                ns_iter_b(L, lanes[L])
        for L in range(2):
            ns_final(L, lanes[L])

        r2lanes = [ns_setup(L) for L in range(2, 4)]
        ytsbs = {}
        HALF = NS_ITERS // 2
        next_mm2 = 0
        for g in range(NG):
            xin = emit_dma_in(g)
            steps = iter(range(4)) if g < HALF else iter(())

            def fill():
                k2 = next(steps, None)
                if k2 is None:
                    return
                for L in range(2):
                    if k2 % 2 == 0:
                        ns_iter_a(2 + L, r2lanes[L],
                                  nc.scalar if L == 0 else nc.vector)
                    else:
                        ns_iter_b(2 + L, r2lanes[L])

            if g < HALF:
                fill()
            elif g == HALF:
                for L in range(2, 4):
                    ns_final(L, r2lanes[L - 2])
            ytsbs[g] = emit_mm1_t1(g, xin, fill)
            fill()
            fill()
            n_emit = 1 if g < NG - 8 else 2
            for _ in range(n_emit):
                if next_mm2 <= g - mm2_delay or (g >= NG - 8
                                                 and next_mm2 <= g):
                    if next_mm2 in ytsbs:
                        emit_mm2(next_mm2, ytsbs.pop(next_mm2))
                        next_mm2 += 1
        while next_mm2 < NG:
            emit_mm2(next_mm2, ytsbs.pop(next_mm2))
            next_mm2 += 1

    return nc


def _unscramble(scr):
    """scr [NG, 128, 2048] bf16 -> [512, 4, 64, 64] f32.
    scr[g, 64b + j', 512c + 64q + i] = Z[16g + 2q + b, c][i, j']."""
    a = np.asarray(scr).astype(np.float32)
    a = a.reshape(NG, 2, 64, HPC, 8, 64)        # g, b, j', c, q, i
    a = a.transpose(0, 4, 1, 3, 5, 2)           # g, q, b, c, i, j'
    return np.ascontiguousarray(a.reshape(B, HPC, H, W))


def _make_in_maps(wkv, p_left, p_right):
    in_maps = []
    for k in range(N_CORES):
        sl = slice(HPC * k, HPC * k + HPC)
        # [b, c, i, j] -> [c, i, b, j], cast bf16
        wt = wkv[:, sl].transpose(1, 2, 0, 3)
        in_maps.append({
            "wkv": wt.astype(BF_NP),
            "p_left": np.ascontiguousarray(p_left[sl]),
            "p_right": np.ascontiguousarray(p_right[sl]),
        })
    return in_maps


_CACHED = {}


def _get_program():
    if "nc" not in _CACHED:
        _CACHED["nc"] = build()
    return _CACHED["nc"]


def kernel(wkv, p_left, p_right):
    wkv = np.ascontiguousarray(wkv, dtype=np.float32)
    p_left = np.ascontiguousarray(p_left, dtype=np.float32)
    p_right = np.ascontiguousarray(p_right, dtype=np.float32)
    assert wkv.shape == (B, 32, H, W), wkv.shape

    nc = _get_program()
    in_maps = _make_in_maps(wkv, p_left, p_right)
    res = run_bass_kernel_spmd(nc, in_maps, list(range(N_CORES)))
    return np.concatenate(
        [_unscramble(np.asarray(res.results[k]["out_scr"]))
         for k in range(N_CORES)], axis=1)


# revision 18
# speedup vs baseline: 1.0986x; 1.0986x over previous
"""BilateralRotation Trainium2 kernel: out[b,c] = R1[c] @ wkv[b,c] @ R2[c],
R = Cayley(p) = (I - A)(I + A)^-1, A = 0.5(p - p^T).

Sharding: 8 NeuronCores, head-parallel - core k owns heads [4k, 4k+4) for all
512 batches. bf16 end-to-end on the data path (rel-err budget 2e-2; measured
~4.5e-3): the host converts the wkv shard to bf16 in a [c, i, b, j] layout so
every DMA line is a 2KB contiguous run, and the device writes bf16 results
that the host casts back to fp32.

Device program per core:
  Phase 1 - Cayley via Newton-Schulz in fp32, 4 pair-packed lanes
    (two 64x64 matrices stacked on 128 partitions; block-diagonal stationaries
    so each NS step is ONE matmul per product):
      B = A^T A, M = I + B, X' = 2X - X(MX), 12 iters, X0 = I/300.
      X is symmetric (polynomial in M), so no X^T tracking is needed.
      R1 path stores R^T = X C^T, R2 path stores R = C X (C^T = I + 2A - B).
  Phase 2 - main loop over 32 groups of 16 batches, all-bf16:
    - MM1: K=128 block-diag stationary blockdiag(R1_c0^T, R1_c1^T) per head
      pair; moving = xin [128, 512] -> Y pair-stacked in PSUM (1 col/cycle,
      full PE).
    - T1: PE transposes of bf16 [128,128] blocks (1 cyc/row vs 2 for fp32).
    - MM2: stationary blockdiag(R2_c, R2_c) (batch-pair K packing), moving
      gathers head c' columns via a strided AP, N=512.
    - PSUM->SBUF copies split across DVE / Act / Pool engines.
    - zsb [128, 2048] bf16 dumped contiguously; host inverts the fixed index
      permutation while unsharding.
"""

import sys
import types
from contextlib import ExitStack

import numpy as np
import ml_dtypes

# ---------------------------------------------------------------------------
# TileContext patch: this walrus build accepts only ONE sync-wait per
# instruction; hoist extra waits onto nops inserted before the instruction.
# ---------------------------------------------------------------------------
import concourse.bass as bass
import concourse.tile as tile
from concourse.vector_clock import ScopedClock
from concourse import masks, mybir
from concourse.bass_utils import run_bass_kernel_spmd

WAIT_LIMIT = 1


def _hoist_extra_waits(nc, inst, hint):
    nops = []
    si = inst.sync_info
    if si is not None and len(si.on_wait) > WAIT_LIMIT:
        extras = si.on_wait[:-WAIT_LIMIT]
        del si.on_wait[:-WAIT_LIMIT]
        for w in extras:
            nop = nc.engines[inst.engine].nop(nofuse=True, hint=hint)
            nsi = nop.ins.sync_info
            if nsi is None:
                nop.ins.sync_info = mybir.SyncInfo(on_wait=[w], on_update=[])
            else:
                nsi.on_wait.append(w)
            nops.append(nop.ins)
    return nops


def _split_waits(nc):
    cur_list = nc.cur_bb.bb.instructions
    for f in nc.m.functions:
        for bb in f.blocks:
            orig = list(bb.instructions)
            if not any(i.sync_info and len(i.sync_info.on_wait) > WAIT_LIMIT
                       for i in orig):
                continue
            new_list = []
            for inst in orig:
                nops = _hoist_extra_waits(nc, inst, "split_wait")
                for nop in nops:
                    if cur_list and cur_list[-1] is nop:
                        cur_list.pop()
                    else:
                        cur_list.remove(nop)
                new_list.extend(nops)
                new_list.append(inst)
            bb.instructions[:] = new_list


def _drain_and_barrier(self, tick_clock, wait_clock):
    nc = self.nc
    _split_waits(nc)
    drain_inst = nc.sync.drain()
    wait_clock.add_sem_waits(drain_inst.ins,
                             ScopedClock({None: tick_clock.global_clock}))
    nops = _hoist_extra_waits(nc, drain_inst.ins, "drain_split_wait")
    if nops:
        insts = nc.cur_bb.bb.instructions
        di = insts.index(drain_inst.ins)
        insts.append(insts.pop(di))
    nc.all_engine_barrier()
    assert self.sems is not None
    popped = nc._tile_sem_poison_stack.pop()
    assert popped is self._sem_poison
    nc.clear_and_free_semaphores(list(self.sems.allocated().values()))
    nc.all_engine_barrier()


tile.TileContext._drain_and_barrier = _drain_and_barrier

# ---------------------------------------------------------------------------
# Program builder
# ---------------------------------------------------------------------------
dt = mybir.dt
F32 = dt.float32
BF16 = dt.bfloat16
BF_NP = np.dtype(ml_dtypes.bfloat16)

HPC = 4                     # heads per core
B = 512
H = W = 64
NG = 32                     # batch groups of 16
GB = 16                     # batches per group
N_CORES = 8
NS_ITERS = 12
C0 = 1.0 / 300.0



def _ecopy(eng, dst, src):
    if hasattr(eng, "tensor_copy"):
        eng.tensor_copy(dst, src)
    else:
        eng.copy(dst, src)


def build(in_bufs=12, out_bufs=3):
    nc = bass.Bass("TRN2", target_bir_lowering=False, debug=False,
                   num_devices=N_CORES)
    # [c, i, b, j] bf16 layout: partition lines are 2KB contiguous runs.
    wkv = nc.dram_tensor("wkv", [HPC, H, B, W], BF16, kind="ExternalInput")
    p_left = nc.dram_tensor("p_left", [HPC, H, H], F32, kind="ExternalInput")
    p_right = nc.dram_tensor("p_right", [HPC, W, W], F32,
                             kind="ExternalInput")
    out = nc.dram_tensor("out_scr", [NG, 128, 2048], BF16,
                         kind="ExternalOutput")

    with tile.TileContext(nc) as tc, ExitStack() as ctx:
        const_pool = ctx.enter_context(tc.tile_pool(name="const", bufs=1))
        bd_pool = ctx.enter_context(tc.tile_pool(name="bd", bufs=1))

        ident = const_pool.tile([128, 128], F32, tag="ident")
        masks.make_identity(nc, ident[:])
        i64 = ident[0:64, 0:64]
        identb = const_pool.tile([128, 128], BF16, tag="identb")
        nc.vector.tensor_copy(identb[:], ident[:])
        # [I64; I64] stacked
        istk = const_pool.tile([128, 64], F32, tag="istk")
        nc.vector.tensor_copy(istk[0:64, :], i64)
        nc.scalar.copy(istk[64:128, :], i64)

        bdl = []
        for P in range(2):
            t = bd_pool.tile([128, 128], BF16, tag=f"bdl{P}")
            nc.gpsimd.memset(t[:], 0.0)
            bdl.append(t)
        bdr = []
        for c in range(HPC):
            t = bd_pool.tile([128, 128], BF16, tag=f"bdr{c}")
            nc.gpsimd.memset(t[:], 0.0)
            bdr.append(t)

        # ---------------- Phase 1: Newton-Schulz Cayley (pair lanes) -------
        # lane 0,1: p_left pairs (R1); lane 2,3: p_right pairs (R2).
        # Quadrant matmuls (tile_position inferred from partition offsets)
        # avoid all block-diagonal staging; psb copies alternate Act/DVE and
        # each iteration is emitted in two halves across all lanes so the
        # in-order PE never sits on one lane's copy latency.
        with ExitStack() as nsctx:
            ns_sb = nsctx.enter_context(tc.tile_pool(name="ns_sb", bufs=2))
            ns_keep = nsctx.enter_context(tc.tile_pool(name="ns_keep",
                                                       bufs=2))
            ns_ps = nsctx.enter_context(
                tc.tile_pool(name="ns_ps", bufs=1, space="PSUM"))

            def qmm(out_t, s, m):
                nc.tensor.matmul(out_t[0:64, :], s[0:64, :], m[0:64, :])
                nc.tensor.matmul(out_t[64:128, :], s[64:128, :],
                                 m[64:128, :])

            lanes = []
            for L in range(4):
                src_t = p_left if L < 2 else p_right
                ca, cb = 2 * (L % 2), 2 * (L % 2) + 1
                pside = ns_sb.tile([64, 128], F32, tag=f"pside{L}")
                nc.sync.dma_start(pside[:, 0:64], src_t.ap()[ca])
                nc.sync.dma_start(pside[:, 64:128], src_t.ap()[cb])
                pstk = ns_sb.tile([128, 64], F32, tag=f"pstk{L}")
                nc.sync.dma_start(pstk[0:64, :], src_t.ap()[ca])
                nc.sync.dma_start(pstk[64:128, :], src_t.ap()[cb])

                ptp = ns_ps.tile([128, 64], F32, tag=f"nsps{L}", bufs=2)
                nc.tensor.transpose(ptp[:], pside[:], i64)

                astk = ns_keep.tile([128, 64], F32, tag=f"astk{L}")
                nc.vector.tensor_sub(astk[:], pstk[:], ptp[:])
                nc.vector.tensor_scalar_mul(astk[:], astk[:], 0.5)

                bps = ns_ps.tile([128, 64], F32, tag=f"nsps{L}", bufs=2)
                qmm(bps, astk, astk)                            # B = A^T A

                mstk = ns_keep.tile([128, 64], F32, tag=f"mstk{L}")
                nc.vector.tensor_add(mstk[:], bps[:], istk[:])  # M = I + B
                ctstk = ns_keep.tile([128, 64], F32, tag=f"ct{L}")
                nc.vector.scalar_tensor_tensor(                 # 2A - B
                    ctstk[:], astk[:], 2.0, bps[:],
                    op0=mybir.AluOpType.mult,
                    op1=mybir.AluOpType.subtract)
                nc.vector.tensor_add(ctstk[:], ctstk[:], istk[:])   # C^T
                xstk = ns_keep.tile([128, 64], F32, tag=f"x{L}")
                nc.vector.tensor_scalar_mul(xstk[:], istk[:], C0)
                lanes.append(dict(xstk=xstk, mstk=mstk, ctstk=ctstk))

            for k in range(NS_ITERS):
                for L, ln in enumerate(lanes):
                    pps = ns_ps.tile([128, 64], F32, tag=f"nsps{L}", bufs=2)
                    qmm(pps, ln["mstk"], ln["xstk"])            # M X (sym)
                    psb = ns_sb.tile([128, 64], F32, tag=f"psb{L}")
                    _ecopy(nc.scalar if L % 2 else nc.vector, psb[:], pps[:])
                    ln["psb"] = psb
                for L, ln in enumerate(lanes):
                    wps = ns_ps.tile([128, 64], F32, tag=f"nsps{L}", bufs=2)
                    qmm(wps, ln["xstk"], ln["psb"])             # X(MX) (sym)
                    xnew = ns_keep.tile([128, 64], F32, tag=f"x{L}")
                    nc.vector.scalar_tensor_tensor(             # 2X - X(MX)
                        xnew[:], ln["xstk"][:], 2.0, wps[:],
                        op0=mybir.AluOpType.mult,
                        op1=mybir.AluOpType.subtract)
                    ln["xstk"] = xnew

            for L, ln in enumerate(lanes):
                rs = ns_ps.tile([128, 64], F32, tag=f"nsps{L}", bufs=2)
                if L < 2:
                    # R^T = X C^T (X symmetric) -> bdl blocks
                    qmm(rs, ln["xstk"], ln["ctstk"])
                    nc.vector.tensor_copy(bdl[L][0:64, 0:64], rs[0:64, :])
                    nc.scalar.copy(bdl[L][64:128, 64:128], rs[64:128, :])
                else:
                    # R = C X = (C^T)^T X -> bdr blocks (replicated)
                    qmm(rs, ln["ctstk"], ln["xstk"])
                    ca, cb = 2 * (L - 2), 2 * (L - 2) + 1
                    nc.vector.tensor_copy(bdr[ca][0:64, 0:64], rs[0:64, :])
                    nc.scalar.copy(bdr[ca][64:128, 64:128], rs[0:64, :])
                    nc.vector.tensor_copy(bdr[cb][0:64, 0:64],
                                          rs[64:128, :])
                    nc.scalar.copy(bdr[cb][64:128, 64:128], rs[64:128, :])

        # ---------------- Phase 2: main loop (all bf16) ----------------
        io_pool = ctx.enter_context(tc.tile_pool(name="io", bufs=in_bufs))
        out_pool = ctx.enter_context(tc.tile_pool(name="outp", bufs=out_bufs))
        mid_pool = ctx.enter_context(tc.tile_pool(name="mid", bufs=2))
        ps_pool = ctx.enter_context(
            tc.tile_pool(name="mainps", bufs=1, space="PSUM"))

        PSTRIDE = B * W
        for g in range(NG):
            xin = []
            for P in range(2):
                t = io_pool.tile([128, 1024], BF16, tag=f"xin{P}")
                off = (2 * P) * H * PSTRIDE + g * GB * W
                nc.sync.dma_start(
                    t[:], bass.AP(wkv, off, [[PSTRIDE, 128], [1, 1024]]))
                xin.append(t)

            ysb = [mid_pool.tile([128, 1024], BF16, tag=f"ysb{P}",
                                 name=f"ysb{P}_{g}") for P in range(2)]
            mm1_eng = ([nc.scalar, nc.scalar, nc.vector, nc.scalar] if g % 2
                       else [nc.scalar, nc.vector, nc.scalar, nc.scalar])
            cp = 0
            for P in range(2):
                for h in range(2):
                    yps = ps_pool.tile([128, 512], F32, tag="mm1", bufs=3)
                    nc.tensor.matmul(yps[:], bdl[P][:],
                                     xin[P][:, 512 * h:512 * h + 512])
                    _ecopy(mm1_eng[cp], ysb[P][:, 512 * h:512 * h + 512],
                           yps[:])
                    cp += 1

            ytsb = [mid_pool.tile([128, 1024], BF16, tag=f"ytsb{P}",
                                  name=f"ytsb{P}_{g}") for P in range(2)]
            for P in range(2):
                tps = ps_pool.tile([128, 1024], BF16, tag="t1", bufs=2)
                for q in range(8):
                    nc.tensor.transpose(
                        tps[:, 128 * q:128 * q + 128],
                        ysb[P][:, 128 * q:128 * q + 128], identb[:])
                nc.vector.tensor_copy(ytsb[P][:], tps[:])

            zsb = out_pool.tile([128, 2048], BF16, tag="zsb")
            mm2_eng = ([nc.vector, nc.scalar, nc.scalar, nc.vector] if g % 2
                       else [nc.vector, nc.scalar, nc.scalar, nc.scalar])
            cp = 0
            for P in range(2):
                for cc in range(2):
                    c = 2 * P + cc
                    zps = ps_pool.tile([128, 512], F32, tag="mm2", bufs=3)
                    base = ytsb[P][:, 64 * cc:64 * cc + 64]
                    rhs = bass.AP(base.tensor, base.offset,
                                  [list(base.ap[0]), [128, 8], [1, 64]])
                    nc.tensor.matmul(zps[:], bdr[c][:], rhs)
                    _ecopy(mm2_eng[cp], zsb[:, 512 * c:512 * c + 512],
                           zps[:])
                    cp += 1

            nc.sync.dma_start(
                bass.AP(out, g * 128 * 2048, [[2048, 128], [1, 2048]]),
                zsb[:])

    return nc


def _unscramble(scr):
    """scr [NG, 128, 2048] bf16 -> [512, 4, 64, 64] f32.
    scr[g, 64b + j', 512c + 64q + i] = Z[16g + 2q + b, c][i, j']."""
    a = np.asarray(scr).astype(np.float32)
    a = a.reshape(NG, 2, 64, HPC, 8, 64)        # g, b, j', c, q, i
    a = a.transpose(0, 4, 1, 3, 5, 2)           # g, q, b, c, i, j'
    return np.ascontiguousarray(a.reshape(B, HPC, H, W))


def _make_in_maps(wkv, p_left, p_right):
    in_maps = []
    for k in range(N_CORES):
        sl = slice(HPC * k, HPC * k + HPC)
        # [b, c, i, j] -> [c, i, b, j], cast bf16
        wt = wkv[:, sl].transpose(1, 2, 0, 3)
        in_maps.append({
            "wkv": wt.astype(BF_NP),
            "p_left": np.ascontiguousarray(p_left[sl]),
            "p_right": np.ascontiguousarray(p_right[sl]),
        })
    return in_maps


_CACHED = {}


def _get_program():
    if "nc" not in _CACHED:
        _CACHED["nc"] = build()
    return _CACHED["nc"]


def kernel(wkv, p_left, p_right):
    wkv = np.ascontiguousarray(wkv, dtype=np.float32)
    p_left = np.ascontiguousarray(p_left, dtype=np.float32)
    p_right = np.ascontiguousarray(p_right, dtype=np.float32)
    assert wkv.shape == (B, 32, H, W), wkv.shape

    nc = _get_program()
    in_maps = _make_in_maps(wkv, p_left, p_right)
    res = run_bass_kernel_spmd(nc, in_maps, list(range(N_CORES)))
    return np.concatenate(
        [_unscramble(np.asarray(res.results[k]["out_scr"]))
         for k in range(N_CORES)], axis=1)
